# revision 12
# baseline (speedup 1.0000x reference)
"""CausalGateUnit Trainium2 kernel — fp8 DoubleRow scores + paired TTR drains.

Math (see reference):
  p_pre = q @ W_pre + b_pre ; p_haz = q @ W_haz + b_haz          [B,S,D]
  gates = sigmoid(q @ W_gate + b_gate)                           [B,S,2]
  sim_x = (p_x @ k^T) * (1/sqrt(D)), strictly-causal masked (j<i)
  score_x[i] = max_j<i sim_x[i,j]   (0 when no visible j, i.e. i==0)
  rs = [g_pre, score_pre, g_haz, score_haz]                      [B,S,4]
  out = relu(rs @ W_s1 + b_s1) @ W_s2 + b_s2                     [B,S,D]

Sharding over 8 cores: core = (b, r) with b = core//4, r = core%4.
Core (b, r) owns row tiles t = 4g + r (g = 0..7) of batch b — 1024 rows.
Slot g computes score chunks over columns [0, 512*(g+1)); every core runs an
identical instruction stream, with the causal diagonal handled by a per-core
fp8 mask tile added via a DoubleRow identity matmul (both planes -> -480).

Score matmuls run in fp8e4m3 with the DoubleRow perf mode (2 contraction
planes per instruction).  TERMS controls a residual-correction scheme:
  1: sim ~= p8.k8                      (fastest, most quantization noise)
  2: sim ~= p8.k8 + pr8.k8             (pT residual, kills pT noise)
  3: sim ~= p8.k8 + pr8.k8 + p8.kr8    (both residuals, ~bf16 accuracy)
p is computed in bf16 (phase A), boosted by PB=2 so the fp8 quantization of
p sits in a good exponent range; the 1/(PB*sqrt(D)) descale is folded into
W_s1 rows 1,3 on the host.

PSUM score-chunk drains are the second wall: the row-max reduce can only
run on DVE (ScalarE has no max, GPSIMD has no PSUM port / no TensorTensor
codegen) at 1 elem/cycle/partition.  Chunks are emitted in pairs into one
[P, 2, CHUNK] psum tile so a single XY-reduce drains two chunks,
amortizing the ~125ns PSUM-access init.
"""

import sys

for _p in ("/opt/trn_rl_repo",):
    if _p not in sys.path:
        sys.path.insert(0, _p)

import numpy as np

B, S, D = 2, 4096, 512
NCORES = 8
P = 128          # partitions / row-tile size
NSLOT = 8        # row tiles per core
ROWS = NSLOT * P  # 1024 rows per core
D1 = 256         # MLP hidden
CHUNK = 512      # score column chunk
CONSTW = 3584    # packed small-constant tile width
KT = D // P      # 4 contraction tiles
NEG = -1.0e30
PB = 2.0         # probe boost folded into Wp/Wh (fp8 exponent placement)
MASKVAL = -240.0  # per plane; DoubleRow adds both planes -> -480
TERMS = 1        # residual terms in the fp8 score matmul (1, 2 or 3)

_PROGRAM_CACHE = {}


def _build_program(with_bias=True, terms=TERMS):
    import concourse.bacc as bacc
    import concourse.mybir as mybir
    import concourse.tile as tile

    f32 = mybir.dt.float32
    bf16 = mybir.dt.bfloat16
    f16 = mybir.dt.float16
    f8 = mybir.dt.float8e4
    AX = mybir.AxisListType
    MAX = mybir.AluOpType.max
    ACT = mybir.ActivationFunctionType
    DR = mybir.MatmulPerfMode.DoubleRow

    nc = bacc.Bacc()

    qT_d = nc.declare_dram_parameter("qT", [D, ROWS], bf16, isOutput=False)
    kT_d = nc.declare_dram_parameter("kT", [D, S], f8, isOutput=False)
    kTr_d = None
    if terms >= 3:
        kTr_d = nc.declare_dram_parameter("kTr", [D, S], f8, isOutput=False)
    Wp_d = nc.declare_dram_parameter("Wp", [D, D], bf16, isOutput=False)
    Wh_d = nc.declare_dram_parameter("Wh", [D, D], bf16, isOutput=False)
    Wg_d = nc.declare_dram_parameter("Wg", [D, 2], bf16, isOutput=False)
    Ws2_d = nc.declare_dram_parameter("Ws2", [D1, D], bf16, isOutput=False)
    cn_d = nc.declare_dram_parameter("consts", [P, CONSTW], bf16, isOutput=False)
    cb_d = nc.declare_dram_parameter("cbf", [P, 2 * (P + CHUNK)], f8, isOutput=False)
    out_d = nc.declare_dram_parameter("out", [ROWS, D], f16, isOutput=True)

    with tile.TileContext(nc) as tc:
        with (
            tc.tile_pool(name="const", bufs=1) as const,
            tc.tile_pool(name="scpart", bufs=6) as spool,
            tc.tile_pool(name="scfin", bufs=4) as fpool,
            tc.tile_pool(name="outs", bufs=3) as opool,
        ):
            kT_sb = const.tile([P, KT, S], f8)
            kTr_sb = (
                const.tile([P, KT, S], f8, name="kTr_sb") if terms >= 3 else None
            )
            qT_sb = const.tile([P, KT, ROWS], bf16)
            Wp_sb = const.tile([P, KT, D], bf16)
            Wh_sb = const.tile([P, KT, D], bf16)
            Wg_sb = const.tile([P, KT, 2], bf16)
            Ws2_sb = const.tile([P, 2, D], bf16)
            consts_sb = const.tile([P, CONSTW], bf16)
            pTp_sb = const.tile([P, KT, ROWS], f8)
            pTh_sb = const.tile([P, KT, ROWS], f8)
            pTpr_sb = (
                const.tile([P, KT, ROWS], f8, name="pTpr_sb") if terms >= 2 else None
            )
            pThr_sb = (
                const.tile([P, KT, ROWS], f8, name="pThr_sb") if terms >= 2 else None
            )
            h1T_sb = const.tile([P, 2, ROWS], bf16)
            rsT = const.tile([5, ROWS], bf16)
            cbf_sb = const.tile([P, 2, P + CHUNK], f8)
            ident = cbf_sb[:, :, 0:P]
            Cm_sb = cbf_sb[:, :, P : P + CHUNK]
            ones = consts_sb[0:1, 640:1664]
            Ws1_sb = consts_sb[0:5, 1664:1920]
            bp_sb = consts_sb[0:1, 1920:2432]
            bh_sb = consts_sb[0:1, 2432:2944]
            bs2_sb = consts_sb[0:1, 2944:3456]
            bg_sb = consts_sb[0:1, 3456:3458]

            # --- constant loads ---
            # qT/Wp/Wh first: phase A can start as soon as they land
            qT_r = qT_d[:, :].rearrange("(t p) n -> p t n", p=P)
            nc.sync.dma_start(out=qT_sb[:, :, 0:CHUNK], in_=qT_r[:, :, 0:CHUNK])
            Wp_r = Wp_d[:, :].rearrange("(t p) n -> p t n", p=P)
            nc.sync.dma_start(out=Wp_sb[:, :, 0:2 * P], in_=Wp_r[:, :, 0:2 * P])
            nc.sync.dma_start(out=Wp_sb[:, :, 2 * P:D], in_=Wp_r[:, :, 2 * P:D])
            nc.sync.dma_start(out=qT_sb[:, :, CHUNK:ROWS], in_=qT_r[:, :, CHUNK:ROWS])
            nc.sync.dma_start(
                out=Wh_sb, in_=Wh_d[:, :].rearrange("(t p) n -> p t n", p=P)
            )
            nc.sync.dma_start(out=consts_sb, in_=cn_d[:, :])
            nc.sync.dma_start(
                out=cbf_sb,
                in_=cb_d[:, :].rearrange("p (t n) -> p t n", t=2),
            )
            nc.sync.dma_start(
                out=Wg_sb, in_=Wg_d[:, :].rearrange("(t p) n -> p t n", p=P)
            )
            nc.sync.dma_start(
                out=Ws2_sb, in_=Ws2_d[:, :].rearrange("(t p) n -> p t n", p=P)
            )
            # kT split by column chunk so slot g only waits on chunks <= g
            kT_r = kT_d[:, :].rearrange("(t p) n -> p t n", p=P)
            kTr_r = None
            if terms >= 3:
                kTr_r = kTr_d[:, :].rearrange("(t p) n -> p t n", p=P)
            for c in range(S // CHUNK):
                cs = slice(c * CHUNK, (c + 1) * CHUNK)
                nc.sync.dma_start(out=kT_sb[:, :, cs], in_=kT_r[:, :, cs])
                if terms >= 3:
                    nc.sync.dma_start(out=kTr_sb[:, :, cs], in_=kTr_r[:, :, cs])

            # compute engines can't start at partition 4; DMA can
            nc.sync.dma_start(out=rsT[4:5, :], in_=ones[0:1, :])

            # psX (2 banks) stays open throughout for gate + MLP psums
            psX = tc.tile_pool(name="psX", bufs=2, space="PSUM")
            psXp = psX.__enter__()
            # PE warmup while input DMAs stream: ~2.3us of dummy matmuls so
            # HAM un-throttles before the real stream starts
            with tc.tile_pool(name="warm", bufs=1, space="PSUM") as warm:
                win = const.tile([P, CHUNK], bf16)
                nc.vector.memset(win, 0.0)
                wps = warm.tile([P, CHUNK], f32, tag="w")
                for _ in range(11):
                    nc.tensor.matmul(
                        wps, lhsT=win[:, 0:P], rhs=win, start=True, stop=True
                    )

            # --- phase A (per n-half): pT = (W^T qT) + b (bf16) -> fp8 ---
            def emit_phase_a(psA, n):
                ns = slice(n * CHUNK, (n + 1) * CHUNK)
                for W_sb, b_sb, pT_sb, pTr_sb in (
                    (Wp_sb, bp_sb, pTp_sb, pTpr_sb),
                    (Wh_sb, bh_sb, pTh_sb, pThr_sb),
                ):
                    for m in range(KT):
                        ms = slice(m * P, (m + 1) * P)
                        ps = psA.tile([P, CHUNK], f32, tag="pt")
                        for kt in range(KT):
                            nc.tensor.matmul(
                                ps,
                                lhsT=W_sb[:, kt, ms],
                                rhs=qT_sb[:, kt, ns],
                                start=(kt == 0),
                                stop=(not with_bias and kt == KT - 1),
                            )
                        if with_bias:
                            nc.tensor.matmul(
                                ps,
                                lhsT=b_sb[0:1, ms],
                                rhs=ones[0:1, ns],
                                start=False,
                                stop=True,
                            )
                        nc.scalar.copy(out=pT_sb[:, m, ns], in_=ps)
                        if terms >= 2:
                            # residual: pr8 = fp8(psum - dequant(p8))
                            nc.vector.scalar_tensor_tensor(
                                out=pTr_sb[:, m, ns],
                                in0=ps,
                                scalar=1.0,
                                in1=pT_sb[:, m, ns],
                                op0=mybir.AluOpType.mult,
                                op1=mybir.AluOpType.subtract,
                            )

            def emit_gates(n):
                ns = slice(n * CHUNK, (n + 1) * CHUNK)
                psg = psXp.tile([2, CHUNK], f32, tag="aux")
                for kt in range(KT):
                    nc.tensor.matmul(
                        psg,
                        lhsT=Wg_sb[:, kt, :],
                        rhs=qT_sb[:, kt, ns],
                        start=(kt == 0),
                        stop=(not with_bias and kt == KT - 1),
                    )
                if with_bias:
                    nc.tensor.matmul(
                        psg,
                        lhsT=bg_sb[0:1, :],
                        rhs=ones[0:1, ns],
                        start=False,
                        stop=True,
                    )
                gt = fpool.tile([2, CHUNK], bf16, tag="gt")
                nc.scalar.activation(out=gt, in_=psg, func=ACT.Sigmoid)
                nc.sync.dma_start(out=rsT[0:1, ns], in_=gt[0:1, :])
                nc.sync.dma_start(out=rsT[2:3, ns], in_=gt[1:2, :])

            # --- phase B: causal scores + row max, MLP fused ---
            def emit_mlp(g):
                # h1 = relu(Ws1_aug.T @ rs), h = h1.T @ Ws2 (+ b_s2)
                gs = slice(g * P, (g + 1) * P)
                ph1 = psXp.tile([P, 2, P], f32, tag="aux", name="ph1")
                for m in range(2):
                    ms = slice(m * P, (m + 1) * P)
                    nc.tensor.matmul(
                        ph1[:, m, :],
                        lhsT=Ws1_sb[0:5, ms],
                        rhs=rsT[0:5, gs],
                        start=True,
                        stop=True,
                    )
                nc.scalar.activation(out=h1T_sb[:, :, gs], in_=ph1, func=ACT.Relu)
                ph = psXp.tile([P, D], f32, tag="aux", name="ph")
                for m in range(2):
                    nc.tensor.matmul(
                        ph,
                        lhsT=h1T_sb[:, m, gs],
                        rhs=Ws2_sb[:, m, :],
                        start=(m == 0),
                        stop=(not with_bias and m == 1),
                    )
                if with_bias:
                    nc.tensor.matmul(
                        ph,
                        lhsT=ones[0:1, 0:P],
                        rhs=bs2_sb[0:1, :],
                        start=False,
                        stop=True,
                    )
                ob = opool.tile([P, D], f16, tag="ob")
                nc.scalar.copy(out=ob, in_=ph)
                nc.sync.dma_start(out=out_d[gs, :], in_=ob)

            def emit_chunk(ps, pT_sb, pTr_sb, gs, c, diag):
                # one 128x512 score chunk: fp8 DoubleRow terms (+ causal mask)
                cs = slice(c * CHUNK, (c + 1) * CHUNK)
                term_ops = [(pT_sb, kT_sb)]
                if terms >= 2:
                    term_ops.append((pTr_sb, kT_sb))
                if terms >= 3:
                    term_ops.append((pT_sb, kTr_sb))
                n_mm = 2 * len(term_ops)
                i = 0
                for lhs_sb, rhs_sb in term_ops:
                    for kt2 in range(2):
                        sl = slice(2 * kt2, 2 * kt2 + 2)
                        nc.tensor.matmul(
                            ps,
                            lhsT=lhs_sb[:, sl, gs],
                            rhs=rhs_sb[:, sl, cs],
                            start=(i == 0),
                            stop=(i == n_mm - 1 and not diag),
                            perf_mode=DR,
                        )
                        i += 1
                if diag:
                    # += 2*Cm (0 where j<i, -240 per plane elsewhere)
                    nc.tensor.matmul(
                        ps,
                        lhsT=ident,
                        rhs=Cm_sb,
                        start=False,
                        stop=True,
                        perf_mode=DR,
                    )

            mlp_pending = []

            def emit_slot(psB, g):
                gs = slice(g * P, (g + 1) * P)
                nch = g + 1
                for pT_sb, pTr_sb, ridx in (
                    (pTp_sb, pTpr_sb, 1),
                    (pTh_sb, pThr_sb, 3),
                ):
                    scp = spool.tile([P, 4], bf16, tag="scp")
                    sct = fpool.tile([P, 1], bf16, tag="sct")
                    idx = 0
                    c = 0
                    while c < nch:
                        w = min(2, nch - c)
                        ps = psB.tile([P, 2, CHUNK], f32, tag="sc")
                        for j in range(w):
                            emit_chunk(
                                ps[:, j, :], pT_sb, pTr_sb, gs, c + j,
                                c + j == g,
                            )
                        red_out = sct if nch <= 2 else scp[:, idx : idx + 1]
                        nc.vector.tensor_reduce(
                            out=red_out,
                            in_=ps[:, 0:w, :],
                            axis=AX.XY,
                            op=MAX,
                        )
                        c += w
                        idx += 1
                    if nch > 2:
                        nc.vector.tensor_reduce(
                            out=sct, in_=scp[:, 0:idx], axis=AX.X, op=MAX
                        )
                    # [128,1] -> [1,128] reorientation
                    nc.sync.dma_start(out=rsT[ridx : ridx + 1, gs], in_=sct)

                mlp_pending.append(g)
                if len(mlp_pending) >= 3:
                    emit_mlp(mlp_pending.pop(0))

            # Interleave: phase A half n=0 -> slots 0..3 (need only pT rows
            # 0..511 and kT chunks 0..3) -> phase A half n=1 -> slots 4..7.
            # DVE drain work starts ~7us in instead of ~30us, and the first
            # phase-B slots don't wait on the tail of the kT DMA stream.
            with tc.tile_pool(name="psB", bufs=2, space="PSUM") as psB:
                with tc.tile_pool(name="psA", bufs=2, space="PSUM") as psA:
                    emit_phase_a(psA, 0)
                    emit_gates(0)
                    emit_gates(1)
                    for g in (0, 1, 2, 3):
                        emit_slot(psB, g)
                    emit_phase_a(psA, 1)
                for g in (4, 5, 6, 7):
                    emit_slot(psB, g)
                for gg in mlp_pending:
                    emit_mlp(gg)
            psX.__exit__(None, None, None)

    nc.compile()
    return nc


def _get_program(debug=False, with_bias=True, terms=TERMS):
    key = ("nc_b" if with_bias else "nc") + f"_t{terms}"
    if key not in _PROGRAM_CACHE:
        _PROGRAM_CACHE[key] = _build_program(with_bias, terms)
    return _PROGRAM_CACHE[key]


def _row_index(r):
    # global row indices (within a batch) owned by core with residue r
    return np.concatenate(
        [np.arange(P) + P * (4 * g + r) for g in range(NSLOT)]
    )


def make_in_maps(q, k, W_pre, b_pre, W_haz, b_haz, W_gate, b_gate, W_s1, b_s1,
                 W_s2, b_s2):
    """Build the 8 per-core input dicts (host-side prep)."""
    import ml_dtypes

    bf = ml_dtypes.bfloat16
    f8 = ml_dtypes.float8_e4m3
    scale = 1.0 / np.sqrt(np.float32(D))
    f = np.float32
    Wp = np.ascontiguousarray((W_pre * PB).astype(f).astype(bf))
    Wh = np.ascontiguousarray((W_haz * PB).astype(f).astype(bf))
    Wg = np.ascontiguousarray(W_gate.astype(f).astype(bf))
    Ws1 = np.concatenate([W_s1.astype(f), b_s1.astype(f).reshape(1, D1)], axis=0)
    Ws1[1, :] *= scale / PB
    Ws1[3, :] *= scale / PB
    Ws2 = np.ascontiguousarray(W_s2.astype(f).astype(bf))

    def packed_consts():
        c = np.zeros((P, CONSTW), f)
        c[0, 640:1664] = 1.0                                # ones
        c[0:5, 1664:1920] = Ws1                             # [5, 256] + b_s1
        c[0, 1920:2432] = (b_pre * PB).astype(f)
        c[0, 2432:2944] = (b_haz * PB).astype(f)
        c[0, 2944:3456] = b_s2.astype(f)
        c[0, 3456:3458] = b_gate.astype(f)
        return c.astype(bf)

    kTb = []
    kTrb = []
    for b in range(B):
        kT_f = np.ascontiguousarray(k[b].T.astype(f))
        k8 = kT_f.astype(f8)
        kTb.append(k8)
        if TERMS >= 3:
            kTrb.append((kT_f - k8.astype(f)).astype(f8))

    def packed_cbf(r):
        c = np.zeros((P, 2, P + CHUNK), f)
        c[:, 0, 0:P] = np.eye(P, dtype=f)
        c[:, 1, 0:P] = np.eye(P, dtype=f)
        pp, ff = np.mgrid[0:P, 0:CHUNK]
        m = np.where(ff < P * r + pp, 0.0, MASKVAL)
        c[:, 0, P : P + CHUNK] = m
        c[:, 1, P : P + CHUNK] = m
        return c.reshape(P, 2 * (P + CHUNK)).astype(f8)

    consts = packed_consts()
    in_maps = []
    for core in range(NCORES):
        b, r = divmod(core, 4)
        rows = _row_index(r)
        qT = np.ascontiguousarray(q[b][rows, :].T.astype(f).astype(bf))
        m = {
            "qT": qT,
            "kT": kTb[b],
            "Wp": Wp,
            "Wh": Wh,
            "Wg": Wg,
            "Ws2": Ws2,
            "consts": consts,
            "cbf": packed_cbf(r),
        }
        if TERMS >= 3:
            m["kTr"] = kTrb[b]
        in_maps.append(m)
    return in_maps


def assemble_output(results, q, W_gate, b_gate, W_s1, b_s1, W_s2, b_s2):
    out = np.empty((B, S, D), np.float32)
    for core in range(NCORES):
        b, r = divmod(core, 4)
        rows = _row_index(r)
        out[b][rows, :] = results[core]["out"].astype(np.float32)
    # row 0 of each batch: no visible keys -> score = 0 (exact host fixup)
    for b in range(B):
        g0 = 1.0 / (1.0 + np.exp(-(q[b, 0].astype(np.float64) @ W_gate + b_gate)))
        rs0 = np.array([g0[0], 0.0, g0[1], 0.0])
        h0 = np.maximum(rs0 @ W_s1 + b_s1, 0.0) @ W_s2 + b_s2
        out[b, 0, :] = h0.astype(np.float32)
    return out


def kernel(**inputs):
    from concourse.bass_utils import run_bass_kernel_spmd

    q = np.asarray(inputs["q"], np.float32)
    k = np.asarray(inputs["k"], np.float32)
    args = dict(
        q=q,
        k=k,
        W_pre=np.asarray(inputs["W_pre"], np.float32),
        b_pre=np.asarray(inputs["b_pre"], np.float32),
        W_haz=np.asarray(inputs["W_haz"], np.float32),
        b_haz=np.asarray(inputs["b_haz"], np.float32),
        W_gate=np.asarray(inputs["W_gate"], np.float32),
        b_gate=np.asarray(inputs["b_gate"], np.float32),
        W_s1=np.asarray(inputs["W_s1"], np.float32),
        b_s1=np.asarray(inputs["b_s1"], np.float32),
        W_s2=np.asarray(inputs["W_s2"], np.float32),
        b_s2=np.asarray(inputs["b_s2"], np.float32),
    )
    zero_bias = all(
        not np.any(args[b_]) for b_ in ("b_pre", "b_haz", "b_gate", "b_s1", "b_s2")
    )
    nc = _get_program(with_bias=not zero_bias)
    in_maps = make_in_maps(**args)
    res = run_bass_kernel_spmd(nc, in_maps, list(range(NCORES)))
    return assemble_output(
        res.results,
        q,
        args["W_gate"],
        args["b_gate"],
        args["W_s1"],
        args["b_s1"],
        args["W_s2"],
        args["b_s2"],
    )
